# revision 35
# baseline (speedup 1.0000x reference)
"""Trainium2 Bass kernel for DiagonalSSMLayer.

Math: y = C_w @ h + D*u  where  h[l] = lam*h[l-1] + (B_w @ u)[l]  (per state
channel, lam = sigmoid(log_lambda)).  The reference computes the causal
exponential-decay convolution via FFT; here it is the exact linear recurrence,
done with the native tensor_tensor_scan (fp32 internal state).

Sharding: 8 cores = (batch b in 0..3) x (sequence half s in 0..1).
Each core gets u[b, s*2048:(s+1)*2048, :] transposed so the contraction dim d
sits on SBUF partitions for both GEMMs.  All GEMM operands are bf16 (full PE
rate, half the HBM traffic, 4x cheaper LDWEIGHTS); PSUM accumulation and the
scan state stay fp32; h and y are written bf16.  Rel err ~5e-3 (gate 2e-2).

Cross-half carry: second-half cores prepend a HALO of the last `HALO`
positions of the first half and scan through it, reconstructing the incoming
state up to lam^HALO (~3e-3) -- below bf16 rounding noise.  First-half cores
get a zero halo (uniform SPMD program).

DMA: TRN2 exposes two hardware DGE queues (SP + ACT) plus gpsimd's software
DGE (runs on the Pool cores).  Measured: a hardware queue dispatches one
packet (= one per-partition contiguous run) every ~19ns REGARDLESS of size,
so throughput is purely packet-size-bound: 2KB runs -> 110 GB/s, 8KB -> 440.
Everything is therefore laid out flat chunk-contiguous per partition: u
chunks move as single 1MB transfers with 8KB runs, y stores accumulate a
full chunk in SBUF [128, KT*LC] and leave as one 8KB-run transfer, weights
are 4KB runs.  A queue round-robins among outstanding transfers (issue order
gives no priority), so the u loads are CHAINED in need-order, each gated on
the previous completing via a tiny dependency-carrying dummy DMA.  The
3-element params ride the software DGE (128 tiny packets each would waste
2.4us of hardware dispatch).

Engine split per 512-col chunk (PE ~7us of matmuls is the bound): DVE runs
both scans from PSUM + even-k y-fuse; ACT drains odd-k C@h from PSUM to SBUF
bf16 and premultiplies D*u; Pool adds the two (all-SBUF).  Pool cannot touch
PSUM, scan/fused-stt lower only on DVE, and Pool's tensor_scalar is a slow
software loop -- hence this exact split.
"""

import numpy as np

B, L, DM, NS = 4, 4096, 1024, 256
HALF = L // 2          # 2048 sequence positions per core
NCORES = 8
LC = 512               # l-chunk (matmul free dim / scan chunk)
NLC = HALF // LC       # 4 main chunks
HALO = 256
KT = DM // 128         # 8 k-tiles (contraction over d)
NT = NS // 128         # 2 n-tiles (state channels)

UHW = KT * HALO        # flat halo columns
UCW = KT * LC          # flat chunk columns

_CACHE = {}


def _build(warm=4):
    from concourse import bacc, tile, mybir

    MULT = mybir.AluOpType.mult
    ADD = mybir.AluOpType.add
    f32 = mybir.dt.float32
    bf16 = mybir.dt.bfloat16

    nc = bacc.Bacc("TRN2", target_bir_lowering=False, debug=False,
                   num_devices=NCORES)

    # flat, chunk-contiguous layouts (big DMA packets)
    uh_d = nc.dram_tensor("uh", [128, UHW], bf16, kind="ExternalInput").ap()
    BwT_d = nc.dram_tensor("BwT", [128, KT * NS], bf16, kind="ExternalInput").ap()
    uC_d = nc.dram_tensor("uC", [128, NLC * UCW], bf16, kind="ExternalInput").ap()
    CwT_d = nc.dram_tensor("CwT", [128, NT * DM], bf16, kind="ExternalInput").ap()
    lam_d = nc.dram_tensor("lamvec", [NS, 1], f32, kind="ExternalInput").ap()
    dvec_d = nc.dram_tensor("dvec", [128, KT], f32, kind="ExternalInput").ap()
    yT_d = nc.dram_tensor("yT", [128, NLC * KT * LC], bf16, kind="ExternalOutput").ap()

    with tile.TileContext(nc) as tc:
        with tc.tile_pool(name="const", bufs=1) as cpool, \
             tc.tile_pool(name="u", bufs=1) as upool, \
             tc.tile_pool(name="h", bufs=1) as hpool, \
             tc.tile_pool(name="y", bufs=4) as ypool, \
             tc.tile_pool(name="bu_ps", bufs=3, space="PSUM") as bupool, \
             tc.tile_pool(name="y_ps", bufs=5, space="PSUM") as yppool:

            # ---- warmup constant + tiny params on the Pool queue
            warm_sb = cpool.tile([128, 512], bf16, name="warm")
            nc.gpsimd.memset(warm_sb[:], 1.0)
            lamv_sb = [cpool.tile([128, 1], f32, name=f"lamv{n}") for n in range(NT)]
            for n in range(NT):
                nc.gpsimd.dma_start(out=lamv_sb[n][:], in_=lam_d[n * 128:(n + 1) * 128, :])
            dvec3 = cpool.tile([128, KT], f32, name="dv")
            nc.gpsimd.dma_start(out=dvec3[:], in_=dvec_d[:, :])
            dvec_sb = [dvec3[:, k:k + 1] for k in range(KT)]

            scr = cpool.tile([128, 16], bf16, name="scr")

            # ---- critical loads: halo + uc0 chained on SP; B_w, then the
            # remaining u chunks, then C_w chained on ACT
            uhf = upool.tile([128, UHW], bf16, name="uhf")
            nc.sync.dma_start(out=uhf[:], in_=uh_d[:, :])
            uh_sb = [uhf[:, k * HALO:(k + 1) * HALO] for k in range(KT)]
            Bwf = cpool.tile([128, KT * NS], bf16, name="bw")
            nc.scalar.dma_start(out=Bwf[:], in_=BwT_d[:, :])
            BwT_sb = [Bwf[:, k * NS:(k + 1) * NS] for k in range(KT)]

            uC_sb = [upool.tile([128, UCW], bf16, name=f"uc{c}")
                     for c in range(NLC)]
            gates = {0: uhf[:, 0:1], 1: Bwf[:, 0:1],
                     2: None, 3: None, 4: None}
            for c in range(NLC):
                eng = nc.sync if c == 0 else nc.scalar
                gate = gates[c] if c < 2 else uC_sb[c - 2][:, 0:1]
                eng.dma_start(out=scr[:, c:c + 1], in_=gate)  # gate
                eng.dma_start(out=uC_sb[c][:],
                              in_=uC_d[:, c * UCW:(c + 1) * UCW])
            CwT3 = cpool.tile([128, NT * DM], bf16, name="cw")
            nc.scalar.dma_start(out=scr[:, 4:5], in_=uC_sb[2][:, 0:1])  # gate
            nc.scalar.dma_start(out=CwT3[:], in_=CwT_d[:, :])
            CwT_sb = [CwT3[:, n * DM:(n + 1) * DM] for n in range(NT)]

            def uC(c, k):
                return uC_sb[c][:, k * LC:(k + 1) * LC]

            # lam broadcast tiles (scans run on DVE; build them there too)
            lam_sb = [cpool.tile([128, LC], f32, name=f"lam{n}") for n in range(NT)]
            for n in range(NT):
                nc.vector.memset(lam_sb[n][:], 1.0)
                nc.vector.tensor_scalar_mul(lam_sb[n][:], lam_sb[n][:], lamv_sb[n][:])

            # ---- PE warmup: dummy matmuls nudge the clock ramp while the
            # halo+Bw DMA streams
            if warm:
                warm_ps = yppool.tile([128, LC], f32, tag="y")
                for w in range(warm):
                    nc.tensor.matmul(warm_ps[:], warm_sb[:, 0:128], warm_sb[:],
                                     start=(w == 0), stop=(w == warm - 1))

            hr = [hpool.tile([128, HALF], bf16, name=f"hr_{n}") for n in range(NT)]
            hh = [hpool.tile([128, HALO], bf16, name=f"hh{n}") for n in range(NT)]

            # ---- halo: GEMM1 + scan over the carry-reconstruction region
            for n in range(NT):
                bu_ps = bupool.tile([128, LC], f32, tag="bu")
                for k in range(KT):
                    nc.tensor.matmul(bu_ps[:, 0:HALO],
                                     BwT_sb[k][:, n * 128:(n + 1) * 128],
                                     uh_sb[k],
                                     start=(k == 0), stop=(k == KT - 1))
                nc.vector.tensor_tensor_scan(
                    hh[n][:], lam_sb[n][:, 0:HALO], bu_ps[:, 0:HALO],
                    0.0, MULT, ADD)

            # ---- main chunks: GEMM1 -> scan -> GEMM2 -> y out.
            # GEMM2/y ops are software-pipelined one chunk behind the scan
            # chain so the next scan never queues behind the previous chunk's
            # y ops on the in-order DVE.
            def gemm2(c):
                o = c * LC
                y8_sb = ypool.tile([128, KT, LC], bf16, tag="ysb")
                for k in range(KT):
                    y_ps = yppool.tile([128, LC], f32, tag="y")
                    for n in range(NT):
                        nc.tensor.matmul(y_ps[:],
                                         CwT_sb[n][:, k * 128:(k + 1) * 128],
                                         hr[n][:, o:o + LC],
                                         start=(n == 0), stop=(n == NT - 1))
                    if k % 2 == 0:
                        nc.vector.scalar_tensor_tensor(
                            y8_sb[:, k, :], uC(c, k),
                            dvec_sb[k], y_ps[:], MULT, ADD)
                    else:
                        ch_sb = ypool.tile([128, LC], bf16, tag="chsb")
                        nc.scalar.copy(ch_sb[:], y_ps[:])
                        ud_sb = ypool.tile([128, LC], bf16, tag="udsb")
                        nc.scalar.mul(ud_sb[:], uC(c, k), dvec_sb[k])
                        nc.gpsimd.tensor_tensor(
                            y8_sb[:, k, :], ud_sb[:], ch_sb[:], ADD)
                base = c * KT * LC
                eng = nc.sync if c % 2 == 0 else nc.scalar
                if c == NLC - 1:
                    # split the last store so draining starts mid-gemm2
                    half = KT // 2 * LC
                    nc.sync.dma_start(out=yT_d[:, base:base + half],
                                      in_=y8_sb[:, 0:KT // 2, :])
                    nc.scalar.dma_start(out=yT_d[:, base + half:base + KT * LC],
                                        in_=y8_sb[:, KT // 2:KT, :])
                else:
                    eng.dma_start(out=yT_d[:, base:base + KT * LC],
                                  in_=y8_sb[:])

            for c in range(NLC):
                o = c * LC
                for n in range(NT):
                    bu_ps = bupool.tile([128, LC], f32, tag="bu")
                    for k in range(KT):
                        nc.tensor.matmul(bu_ps[:],
                                         BwT_sb[k][:, n * 128:(n + 1) * 128],
                                         uC(c, k),
                                         start=(k == 0), stop=(k == KT - 1))
                    init = (hh[n][:, HALO - 1:HALO] if c == 0
                            else hr[n][:, o - 1:o])
                    nc.vector.tensor_tensor_scan(
                        hr[n][:, o:o + LC],
                        lam_sb[n][:], bu_ps[:], init, MULT, ADD)
                if c > 0:
                    gemm2(c - 1)
            gemm2(NLC - 1)

    nc.compile()
    return nc


def _sigmoid(x):
    return 1.0 / (1.0 + np.exp(-x))


def kernel(u, log_lambda, B_w, C_w, D):
    import ml_dtypes
    from concourse.bass_utils import run_bass_kernel_spmd

    bf16 = ml_dtypes.bfloat16

    if "nc" not in _CACHE:
        _CACHE["nc"] = _build()
    nc = _CACHE["nc"]

    lam = _sigmoid(np.asarray(log_lambda, dtype=np.float64))
    # [128, KT*N] flat k-major layouts: row p of k-block k holds d = k*128+p
    BwT = np.ascontiguousarray(
        np.asarray(B_w, np.float32).T.reshape(KT, 128, NS)
        .transpose(1, 0, 2).reshape(128, KT * NS)).astype(bf16)
    CwT = np.ascontiguousarray(
        np.asarray(C_w, np.float32).T.reshape(NT, 128, DM)
        .transpose(1, 0, 2).reshape(128, NT * DM)).astype(bf16)
    dvec = np.ascontiguousarray(np.asarray(D, np.float32).reshape(KT, 128).T)
    lamvec = np.ascontiguousarray(lam.reshape(NS, 1)).astype(np.float32)

    ub = np.asarray(u, dtype=np.float32).astype(bf16)

    def flat_cols(blk):  # [cols, DM] -> [128, KT*cols], chunk-contiguous
        cols = blk.shape[0]
        return (blk.T.reshape(KT, 128, cols).transpose(1, 0, 2)
                .reshape(128, KT * cols))

    in_maps = []
    for core in range(NCORES):
        b, s = core // 2, core % 2
        lo = s * HALF
        uh = np.zeros((128, UHW), dtype=bf16)
        if s == 1:
            uh[:] = flat_cols(ub[b, lo - HALO:lo, :])
        uCf = np.empty((128, NLC * UCW), dtype=bf16)
        for c in range(NLC):
            uCf[:, c * UCW:(c + 1) * UCW] = flat_cols(
                ub[b, lo + c * LC:lo + (c + 1) * LC, :])
        in_maps.append({
            "uh": uh,
            "BwT": BwT,
            "uC": uCf,
            "CwT": CwT,
            "lamvec": lamvec,
            "dvec": dvec,
        })
    _CACHE["in_maps"] = in_maps

    def _run():
        return run_bass_kernel_spmd(nc, in_maps, core_ids=list(range(NCORES)))

    try:
        res = _run()
    except Exception:
        # a previously failed execution can wedge the backend; reset + retry
        try:
            import ctypes, jax
            jax.devices()
            lib = ctypes.CDLL("/opt/axon/libaxon_pjrt.so")
            lib.axon_reset.restype = ctypes.c_int64
            lib.axon_reset()
        except Exception:
            pass
        res = _run()

    y = np.empty((B, L, DM), dtype=np.float32)
    for core in range(NCORES):
        b, s = core // 2, core % 2
        # yT flat [128, c, k, l] -> y[b, lo + c*LC + l, k*128 + p]
        yt = res.results[core]["yT"].reshape(128, NLC, KT, LC).astype(np.float32)
        y[b, s * HALF:(s + 1) * HALF, :] = (
            yt.transpose(1, 3, 2, 0).reshape(HALF, DM))
    return y
